# revision 6
# baseline (speedup 1.0000x reference)
"""MetaQDA fixed-shot head — Trainium2 Bass kernel (8 NeuronCores, SPMD).

Math: the reference builds per-class covariances
    sigma_c = (L L^T + X_c^T X_c / S + g * dm_c dm_c^T) / r
(rank-6 update of the shared scatter L L^T), inverts all 64 of them and
computes Mahalanobis distances for 2048 queries.  Via the Woodbury identity
the whole query-side computation collapses to a single fused matmul
    P = X_query @ Wbig          Wbig: [D, D + C + 6C] = [512, 960]
followed by cheap per-row reductions:
    dist/sp = rowsum(P[:, :512]^2) + P[:, 512:576] + k_c - group6sum(P[:, 576:]^2)
    out     = biases_c - 0.5 (sp + D) * log(1 + dist/sp)
The O(D^3 + C D^2) one-time setup (one triangular inverse + 64 6x6 inverses,
a few ms of fp64 numpy) runs on host; the O(Q D^2) query work runs on the
NeuronCores, sharded over the query axis (256 queries per core).
"""

import math
import os

import numpy as np

D = 512
C = 64
S = 5
Q = 2048
FIX_NJ = 5.0
NCORES = 8
QLOC = Q // NCORES          # 256 queries per core
NW = D + C + 6 * C          # 960 fused weight columns
RANK = 6


# --------------------------------------------------------------------------
# Host-side one-time setup (fp64): Woodbury factorization of the 64 sigmas.
# --------------------------------------------------------------------------
def _host_precompute(X_support, m, kappa, nu, triu_S_diag, triu_S_lower):
    m = np.asarray(m, np.float64).reshape(1, D)
    kappa = float(np.asarray(kappa))
    nu = float(np.asarray(nu))
    diag = np.abs(np.asarray(triu_S_diag, np.float64))
    Lmat = np.diag(diag) + np.asarray(triu_S_lower, np.float64) * np.tril(
        np.ones((D, D)), -1
    )
    kappa_n = abs(kappa) + 1e-6 + FIX_NJ
    m_w = abs(kappa + 1e-6) / kappa_n * m
    xw = FIX_NJ / kappa_n
    gamma = (abs(kappa) + 1e-6) / kappa_n
    sp = max(nu, D - 1 + 1e-6) + FIX_NJ - D + 2
    bias_shared = (
        math.lgamma(0.5 * (sp + D)) - math.lgamma(0.5 * sp) - 0.5 * D * math.log(sp)
    )
    r = (kappa_n + 1) / (kappa_n * sp)               # sigma = stuff / r

    Xc = np.asarray(X_support, np.float64).reshape(C, S, D)
    x_mean = Xc.mean(axis=1)                         # [C,D]
    mu = m_w + x_mean * xw                           # [C,D]
    dm = x_mean - m                                  # [C,D]

    # stuff_c = L L^T + U_c U_c^T with U_c = [X_c^T/sqrt(S) | sqrt(g) dm_c]
    U = np.concatenate(
        [Xc.transpose(0, 2, 1) / np.sqrt(S), np.sqrt(gamma) * dm[:, :, None]], axis=2
    )                                                # [C,D,6]
    Linv = np.linalg.inv(Lmat)
    G = Linv.T @ Linv                                # (L L^T)^{-1}
    logdetA = 2 * np.sum(np.log(diag))

    W = np.einsum("de,cek->cdk", G, U)               # [C,D,6]
    M = np.eye(RANK)[None] + np.einsum("cdk,cdl->ckl", U, W)
    Minv = np.linalg.inv(M)
    _, logdetM = np.linalg.slogdet(M)
    logdet_sigma = logdetA + logdetM - D * np.log(r)
    biases = bias_shared - 0.5 * logdet_sigma        # [C]

    g_vec = mu @ G                                   # [C,D]
    b = np.einsum("cdk,cd->ck", U, g_vec)            # [C,6]
    Minv_b = np.einsum("ckl,cl->ck", Minv, b)
    h = -2 * mu + 2 * np.einsum("cdk,ck->cd", U, Minv_b)   # [C,D]
    k_c = np.einsum("cd,cd->c", mu, g_vec) - np.einsum("ck,ck->c", b, Minv_b)
    N = np.linalg.cholesky(Minv)                     # Minv = N N^T
    V = np.einsum("cdk,ckl->cdl", U, N)              # [C,D,6]

    scale = r / sp
    W1 = Linv.T * np.sqrt(scale)
    W2 = (G @ h.T) * scale
    W3 = np.einsum("de,cek->cdk", G, V).transpose(1, 0, 2).reshape(D, C * RANK)
    W3 = W3 * np.sqrt(scale)
    Wbig = np.concatenate([W1, W2, W3], axis=1)      # [D, 960]
    const_row = 1.0 + scale * k_c                    # [C]
    out_scale = -0.5 * (sp + D)
    return (
        np.ascontiguousarray(Wbig, dtype=np.float32),
        np.ascontiguousarray(const_row, dtype=np.float32),
        np.ascontiguousarray(biases, dtype=np.float32),
        float(out_scale),
    )


# --------------------------------------------------------------------------
# Bass kernel: per core, P = XqT.T @ Wbig then fused reductions + log.
# --------------------------------------------------------------------------
def _build_bass(out_scale):
    import concourse.tile as tile
    from concourse import bacc, mybir

    f32 = mybir.dt.float32
    nc = bacc.Bacc("TRN2", target_bir_lowering=False, debug=False)
    # xqT slice and Wbig fused into one input so each K-chunk is a single
    # DMA (matmul LDWEIGHTS allows only one semaphore wait).
    inp = nc.declare_dram_parameter("inp", [D, QLOC + NW], f32, isOutput=False)
    cb = nc.declare_dram_parameter("cb", [128, 2 * C], f32, isOutput=False)
    out = nc.declare_dram_parameter("out", [QLOC, C], f32, isOutput=True)

    KC = D // 128               # 4 contraction chunks
    QT = QLOC // 128            # 2 query tiles per core
    N_SPLITS = ((0, 512), (512, NW - 512))

    with tile.TileContext(nc) as tc:
        with (
            tc.tile_pool(name="weights", bufs=1) as wpool,
            tc.tile_pool(name="acts", bufs=2) as apool,
            tc.tile_pool(name="scratch", bufs=2) as spool,
            tc.tile_pool(name="psum", bufs=2, space="PSUM") as ppool,
        ):
            cb_sb = wpool.tile([128, 2 * C], f32, tag="cb")
            nc.sync.dma_start(out=cb_sb[:], in_=cb[:, :])
            in_sb = []
            for c in range(KC):
                t = wpool.tile([128, QLOC + NW], f32, tag=f"in{c}")
                nc.sync.dma_start(out=t[:], in_=inp[c * 128 : (c + 1) * 128, :])
                in_sb.append(t)

            for qt in range(QT):
                ps = ppool.tile([128, NW], f32, tag="ps")
                for nlo, nsz in N_SPLITS:
                    for c in range(KC):
                        nc.tensor.matmul(
                            ps[:, nlo : nlo + nsz],
                            in_sb[c][:, qt * 128 : (qt + 1) * 128],
                            in_sb[c][:, QLOC + nlo : QLOC + nlo + nsz],
                            start=(c == 0),
                            stop=(c == KC - 1),
                        )

                # t1 = rowsum(P[:, :512]^2)  (ScalarE: single PSUM input + accum)
                sq = spool.tile([128, D], f32, tag="sq")
                t1 = spool.tile([128, 1], f32, tag="t1")
                nc.scalar.activation(
                    out=sq[:],
                    in_=ps[:, 0:D],
                    func=mybir.ActivationFunctionType.Square,
                    accum_out=t1[:],
                )
                # s2 = group-of-6 rowsum(P[:, 576:960]^2)
                sq6 = spool.tile([128, C * RANK], f32, tag="sq6")
                nc.scalar.activation(
                    out=sq6[:],
                    in_=ps[:, D + C : NW],
                    func=mybir.ActivationFunctionType.Square,
                )
                s2 = spool.tile([128, C], f32, tag="s2")
                nc.vector.reduce_sum(
                    out=s2[:],
                    in_=sq6[:].rearrange("p (c s) -> p c s", s=RANK),
                    axis=mybir.AxisListType.X,
                )
                # u = T2 + const - s2 ; lg = ln(u + t1)
                u = spool.tile([128, C], f32, tag="u")
                nc.vector.tensor_sub(u[:], ps[:, D : D + C], s2[:])
                nc.vector.tensor_add(u[:], u[:], cb_sb[:, 0:C])
                lg = spool.tile([128, C], f32, tag="lg")
                nc.scalar.activation(
                    out=lg[:],
                    in_=u[:],
                    func=mybir.ActivationFunctionType.Ln,
                    bias=t1[:, 0:1],
                    scale=1.0,
                )
                # out = biases + out_scale * lg
                ot = apool.tile([128, C], f32, tag="ot")
                nc.vector.scalar_tensor_tensor(
                    out=ot[:],
                    in0=lg[:],
                    scalar=float(out_scale),
                    in1=cb_sb[:, C : 2 * C],
                    op0=mybir.AluOpType.mult,
                    op1=mybir.AluOpType.add,
                )
                nc.sync.dma_start(
                    out=out[qt * 128 : (qt + 1) * 128, :], in_=ot[:]
                )
    nc.compile()
    return nc


def kernel(X_support, y, X_query, m, kappa, nu, triu_S_diag, triu_S_lower):
    from concourse.bass_utils import run_bass_kernel_spmd

    Wbig, const_row, biases, out_scale = _host_precompute(
        X_support, m, kappa, nu, triu_S_diag, triu_S_lower
    )
    Xq = np.ascontiguousarray(np.asarray(X_query, np.float32))
    XqT = np.ascontiguousarray(Xq.T)                 # [D, Q]
    cb_row = np.concatenate([const_row, biases])     # [2C]
    cb = np.ascontiguousarray(
        np.broadcast_to(cb_row[None, :], (128, 2 * C)), dtype=np.float32
    )

    in_maps = [
        {
            "inp": np.ascontiguousarray(
                np.concatenate([XqT[:, i * QLOC : (i + 1) * QLOC], Wbig], axis=1)
            ),
            "cb": cb,
        }
        for i in range(NCORES)
    ]
    nc = _build_bass(out_scale)
    trace = bool(int(os.environ.get("KBENCH_TRACE", "0")))
    res = run_bass_kernel_spmd(
        nc, in_maps, core_ids=list(range(NCORES)), trace=trace
    )
    if trace:
        kernel.last_exec_time_ns = res.exec_time_ns
        kernel.last_results = res
    out = np.concatenate([res.results[i]["out"] for i in range(NCORES)], axis=0)
    return out


# revision 9
# speedup vs baseline: 1.2159x; 1.2159x over previous
"""MetaQDA fixed-shot head — Trainium2 Bass kernel (8 NeuronCores, SPMD).

Math: the reference builds per-class covariances
    sigma_c = (L L^T + X_c^T X_c / S + g * dm_c dm_c^T) / r
(rank-6 update of the shared scatter L L^T), inverts all 64 of them and
computes Mahalanobis distances for 2048 queries.  Via the Woodbury identity
the whole query-side computation collapses to a single fused matmul
    P = X_query @ Wbig          Wbig: [D, D + C + 6C] = [512, 960]
followed by cheap per-row reductions:
    dist/sp = rowsum(P[:, :512]^2) + P[:, 512:576] + k_c - group6sum(P[:, 576:]^2)
    out     = biases_c - 0.5 (sp + D) * log(1 + dist/sp)
The O(D^3 + C D^2) one-time setup (one triangular inverse + 64 6x6 inverses,
a few ms of fp64 numpy) runs on host; the O(Q D^2) query work runs on the
NeuronCores, sharded over the query axis (256 queries per core).
"""

import math
import os

import numpy as np

D = 512
C = 64
S = 5
Q = 2048
FIX_NJ = 5.0
NCORES = 8
QLOC = Q // NCORES          # 256 queries per core
NW = D + C + 6 * C          # 960 fused weight columns
RANK = 6


# --------------------------------------------------------------------------
# Host-side one-time setup (fp64): Woodbury factorization of the 64 sigmas.
# --------------------------------------------------------------------------
def _host_precompute(X_support, m, kappa, nu, triu_S_diag, triu_S_lower):
    m = np.asarray(m, np.float64).reshape(1, D)
    kappa = float(np.asarray(kappa))
    nu = float(np.asarray(nu))
    diag = np.abs(np.asarray(triu_S_diag, np.float64))
    Lmat = np.diag(diag) + np.asarray(triu_S_lower, np.float64) * np.tril(
        np.ones((D, D)), -1
    )
    kappa_n = abs(kappa) + 1e-6 + FIX_NJ
    m_w = abs(kappa + 1e-6) / kappa_n * m
    xw = FIX_NJ / kappa_n
    gamma = (abs(kappa) + 1e-6) / kappa_n
    sp = max(nu, D - 1 + 1e-6) + FIX_NJ - D + 2
    bias_shared = (
        math.lgamma(0.5 * (sp + D)) - math.lgamma(0.5 * sp) - 0.5 * D * math.log(sp)
    )
    r = (kappa_n + 1) / (kappa_n * sp)               # sigma = stuff / r

    Xc = np.asarray(X_support, np.float64).reshape(C, S, D)
    x_mean = Xc.mean(axis=1)                         # [C,D]
    mu = m_w + x_mean * xw                           # [C,D]
    dm = x_mean - m                                  # [C,D]

    # stuff_c = L L^T + U_c U_c^T with U_c = [X_c^T/sqrt(S) | sqrt(g) dm_c]
    U = np.concatenate(
        [Xc.transpose(0, 2, 1) / np.sqrt(S), np.sqrt(gamma) * dm[:, :, None]], axis=2
    )                                                # [C,D,6]
    Linv = np.linalg.inv(Lmat)
    G = Linv.T @ Linv                                # (L L^T)^{-1}
    logdetA = 2 * np.sum(np.log(diag))

    W = np.einsum("de,cek->cdk", G, U)               # [C,D,6]
    M = np.eye(RANK)[None] + np.einsum("cdk,cdl->ckl", U, W)
    Minv = np.linalg.inv(M)
    _, logdetM = np.linalg.slogdet(M)
    logdet_sigma = logdetA + logdetM - D * np.log(r)
    biases = bias_shared - 0.5 * logdet_sigma        # [C]

    g_vec = mu @ G                                   # [C,D]
    b = np.einsum("cdk,cd->ck", U, g_vec)            # [C,6]
    Minv_b = np.einsum("ckl,cl->ck", Minv, b)
    h = -2 * mu + 2 * np.einsum("cdk,ck->cd", U, Minv_b)   # [C,D]
    k_c = np.einsum("cd,cd->c", mu, g_vec) - np.einsum("ck,ck->c", b, Minv_b)
    N = np.linalg.cholesky(Minv)                     # Minv = N N^T
    V = np.einsum("cdk,ckl->cdl", U, N)              # [C,D,6]

    scale = r / sp
    W1 = Linv.T * np.sqrt(scale)
    W2 = (G @ h.T) * scale
    W3 = np.einsum("de,cek->cdk", G, V).transpose(1, 0, 2).reshape(D, C * RANK)
    W3 = W3 * np.sqrt(scale)
    Wbig = np.concatenate([W1, W2, W3], axis=1)      # [D, 960]
    const_row = 1.0 + scale * k_c                    # [C]
    out_scale = -0.5 * (sp + D)
    return (
        np.ascontiguousarray(Wbig, dtype=np.float32),
        np.ascontiguousarray(const_row, dtype=np.float32),
        np.ascontiguousarray(biases, dtype=np.float32),
        float(out_scale),
    )


# --------------------------------------------------------------------------
# Bass kernel: per core, P = XqT.T @ Wbig then fused reductions + log.
# --------------------------------------------------------------------------
def _build_bass(out_scale):
    import concourse.tile as tile
    from concourse import bacc, mybir

    f32 = mybir.dt.float32
    f32r = mybir.dt.float32r
    nc = bacc.Bacc("TRN2", target_bir_lowering=False, debug=False)
    # xqT slice and Wbig fused into one input so each K-chunk is a single
    # DMA (matmul LDWEIGHTS allows only one semaphore wait).
    inp = nc.declare_dram_parameter("inp", [D, QLOC + NW], f32r, isOutput=False)
    cb = nc.declare_dram_parameter("cb", [128, 2 * C], f32, isOutput=False)
    out = nc.declare_dram_parameter("out", [QLOC, C], f32, isOutput=True)

    KC = D // 128               # 4 contraction chunks
    QT = QLOC // 128            # 2 query tiles per core
    N_SPLITS = ((0, 512), (512, NW - 512))

    with tile.TileContext(nc) as tc:
        with (
            tc.tile_pool(name="weights", bufs=1) as wpool,
            tc.tile_pool(name="acts", bufs=2) as apool,
            tc.tile_pool(name="scratch", bufs=2) as spool,
            tc.tile_pool(name="psum", bufs=2, space="PSUM") as ppool,
        ):
            cb_sb = wpool.tile([128, 2 * C], f32, tag="cb")
            nc.sync.dma_start(out=cb_sb[:], in_=cb[:, :])
            in_sb = []
            for c in range(KC):
                t = wpool.tile([128, QLOC + NW], f32r, tag=f"in{c}")
                nc.sync.dma_start(out=t[:], in_=inp[c * 128 : (c + 1) * 128, :])
                in_sb.append(t)

            for qt in range(QT):
                ps = ppool.tile([128, NW], f32, tag="ps")
                for nlo, nsz in N_SPLITS:
                    for c in range(KC):
                        nc.tensor.matmul(
                            ps[:, nlo : nlo + nsz],
                            in_sb[c][:, qt * 128 : (qt + 1) * 128],
                            in_sb[c][:, QLOC + nlo : QLOC + nlo + nsz],
                            start=(c == 0),
                            stop=(c == KC - 1),
                        )

                # t1 = rowsum(P[:, :512]^2)  (ScalarE: single PSUM input + accum)
                sq = spool.tile([128, D], f32, tag="sq")
                t1 = spool.tile([128, 1], f32, tag="t1")
                nc.scalar.activation(
                    out=sq[:],
                    in_=ps[:, 0:D],
                    func=mybir.ActivationFunctionType.Square,
                    accum_out=t1[:],
                )
                # s2 = group-of-6 rowsum(P[:, 576:960]^2)
                sq6 = spool.tile([128, C * RANK], f32, tag="sq6")
                nc.scalar.activation(
                    out=sq6[:],
                    in_=ps[:, D + C : NW],
                    func=mybir.ActivationFunctionType.Square,
                )
                s2 = spool.tile([128, C], f32, tag="s2")
                nc.vector.reduce_sum(
                    out=s2[:],
                    in_=sq6[:].rearrange("p (c s) -> p c s", s=RANK),
                    axis=mybir.AxisListType.X,
                )
                # u = T2 + const - s2 ; lg = ln(u + t1)
                u = spool.tile([128, C], f32, tag="u")
                nc.vector.tensor_sub(u[:], ps[:, D : D + C], s2[:])
                nc.vector.tensor_add(u[:], u[:], cb_sb[:, 0:C])
                lg = spool.tile([128, C], f32, tag="lg")
                nc.scalar.activation(
                    out=lg[:],
                    in_=u[:],
                    func=mybir.ActivationFunctionType.Ln,
                    bias=t1[:, 0:1],
                    scale=1.0,
                )
                # out = biases + out_scale * lg
                ot = apool.tile([128, C], f32, tag="ot")
                nc.vector.scalar_tensor_tensor(
                    out=ot[:],
                    in0=lg[:],
                    scalar=float(out_scale),
                    in1=cb_sb[:, C : 2 * C],
                    op0=mybir.AluOpType.mult,
                    op1=mybir.AluOpType.add,
                )
                nc.sync.dma_start(
                    out=out[qt * 128 : (qt + 1) * 128, :], in_=ot[:]
                )
    nc.compile()
    return nc


def kernel(X_support, y, X_query, m, kappa, nu, triu_S_diag, triu_S_lower):
    from concourse.bass_utils import run_bass_kernel_spmd

    Wbig, const_row, biases, out_scale = _host_precompute(
        X_support, m, kappa, nu, triu_S_diag, triu_S_lower
    )
    Xq = np.ascontiguousarray(np.asarray(X_query, np.float32))
    XqT = np.ascontiguousarray(Xq.T)                 # [D, Q]
    cb_row = np.concatenate([const_row, biases])     # [2C]
    cb = np.ascontiguousarray(
        np.broadcast_to(cb_row[None, :], (128, 2 * C)), dtype=np.float32
    )

    in_maps = [
        {
            "inp": np.ascontiguousarray(
                np.concatenate([XqT[:, i * QLOC : (i + 1) * QLOC], Wbig], axis=1)
            ),
            "cb": cb,
        }
        for i in range(NCORES)
    ]
    nc = _build_bass(out_scale)
    trace = bool(int(os.environ.get("KBENCH_TRACE", "0")))
    res = run_bass_kernel_spmd(
        nc, in_maps, core_ids=list(range(NCORES)), trace=trace
    )
    if trace:
        kernel.last_exec_time_ns = res.exec_time_ns
        kernel.last_results = res
    out = np.concatenate([res.results[i]["out"] for i in range(NCORES)], axis=0)
    return out


# revision 10
# speedup vs baseline: 1.3955x; 1.1476x over previous
"""MetaQDA fixed-shot head — Trainium2 Bass kernel (8 NeuronCores, SPMD).

Math: the reference builds per-class covariances
    sigma_c = (L L^T + X_c^T X_c / S + g * dm_c dm_c^T) / r
(rank-6 update of the shared scatter L L^T), inverts all 64 of them and
computes Mahalanobis distances for 2048 queries.  Via the Woodbury identity
the whole query-side computation collapses to a single fused matmul
    P = X_query @ Wbig          Wbig: [D, D + C + 6C] = [512, 960]
followed by cheap per-row reductions:
    dist/sp = rowsum(P[:, :512]^2) + P[:, 512:576] + k_c - group6sum(P[:, 576:]^2)
    out     = biases_c - 0.5 (sp + D) * log(1 + dist/sp)
The O(D^3 + C D^2) one-time setup (one triangular inverse + 64 6x6 inverses,
a few ms of fp64 numpy) runs on host; the O(Q D^2) query work runs on the
NeuronCores, sharded over the query axis (256 queries per core).

Device-side details:
 - W1 = sqrt(r/sp) L^{-T} is always upper triangular (L is lower triangular
   by construction), so the strictly-lower 128x128 blocks are skipped in both
   the DMA and the matmuls.  Input is packed per K-chunk: [XqT | W1 | W2W3].
 - Matmuls run as float32r (fp32 bits through the fast PE weight path).
 - A few garbage fp32 matmuls at kernel start keep the PE busy during the
   input DMA so the HAM clock-gate is released (1.2 -> 2.4 GHz) before the
   real matmuls issue.
"""

import math
import os

import numpy as np

D = 512
C = 64
S = 5
Q = 2048
FIX_NJ = 5.0
NCORES = 8
QLOC = Q // NCORES          # 256 queries per core
NW = D + C + 6 * C          # 960 fused weight columns
NB = C + 6 * C              # 448 non-triangular columns (W2 | W3)
RANK = 6
KC = D // 128               # 4 contraction chunks
QT = QLOC // 128            # 2 query tiles per core
# per-chunk packed widths: xq (QLOC) + W1 cols >= 128c + W2W3 (448)
CHUNK_W = [QLOC + (D - 128 * c) + NB for c in range(KC)]
CHUNK_OFF = [128 * sum(CHUNK_W[:c]) for c in range(KC)]
INP_TOTAL = 128 * sum(CHUNK_W)
N_WARM = 5                  # dummy fp32 matmuls to warm the PE clock gate


# --------------------------------------------------------------------------
# Host-side one-time setup (fp64): Woodbury factorization of the 64 sigmas.
# --------------------------------------------------------------------------
def _host_precompute(X_support, m, kappa, nu, triu_S_diag, triu_S_lower):
    m = np.asarray(m, np.float64).reshape(1, D)
    kappa = float(np.asarray(kappa))
    nu = float(np.asarray(nu))
    diag = np.abs(np.asarray(triu_S_diag, np.float64))
    Lmat = np.diag(diag) + np.asarray(triu_S_lower, np.float64) * np.tril(
        np.ones((D, D)), -1
    )
    kappa_n = abs(kappa) + 1e-6 + FIX_NJ
    m_w = abs(kappa + 1e-6) / kappa_n * m
    xw = FIX_NJ / kappa_n
    gamma = (abs(kappa) + 1e-6) / kappa_n
    sp = max(nu, D - 1 + 1e-6) + FIX_NJ - D + 2
    bias_shared = (
        math.lgamma(0.5 * (sp + D)) - math.lgamma(0.5 * sp) - 0.5 * D * math.log(sp)
    )
    r = (kappa_n + 1) / (kappa_n * sp)               # sigma = stuff / r

    Xc = np.asarray(X_support, np.float64).reshape(C, S, D)
    x_mean = Xc.mean(axis=1)                         # [C,D]
    mu = m_w + x_mean * xw                           # [C,D]
    dm = x_mean - m                                  # [C,D]

    # stuff_c = L L^T + U_c U_c^T with U_c = [X_c^T/sqrt(S) | sqrt(g) dm_c]
    U = np.concatenate(
        [Xc.transpose(0, 2, 1) / np.sqrt(S), np.sqrt(gamma) * dm[:, :, None]], axis=2
    )                                                # [C,D,6]
    Linv = np.linalg.inv(Lmat)
    G = Linv.T @ Linv                                # (L L^T)^{-1}
    logdetA = 2 * np.sum(np.log(diag))

    W = np.einsum("de,cek->cdk", G, U)               # [C,D,6]
    M = np.eye(RANK)[None] + np.einsum("cdk,cdl->ckl", U, W)
    Minv = np.linalg.inv(M)
    _, logdetM = np.linalg.slogdet(M)
    logdet_sigma = logdetA + logdetM - D * np.log(r)
    biases = bias_shared - 0.5 * logdet_sigma        # [C]

    g_vec = mu @ G                                   # [C,D]
    b = np.einsum("cdk,cd->ck", U, g_vec)            # [C,6]
    Minv_b = np.einsum("ckl,cl->ck", Minv, b)
    h = -2 * mu + 2 * np.einsum("cdk,ck->cd", U, Minv_b)   # [C,D]
    k_c = np.einsum("cd,cd->c", mu, g_vec) - np.einsum("ck,ck->c", b, Minv_b)
    N = np.linalg.cholesky(Minv)                     # Minv = N N^T
    V = np.einsum("cdk,ckl->cdl", U, N)              # [C,D,6]

    scale = r / sp
    W1 = Linv.T * np.sqrt(scale)                     # [D,D] upper triangular
    W2 = (G @ h.T) * scale                           # [D,C]
    W3 = np.einsum("de,cek->cdk", G, V).transpose(1, 0, 2).reshape(D, C * RANK)
    W3 = W3 * np.sqrt(scale)                         # [D,6C]
    W23 = np.concatenate([W2, W3], axis=1)           # [D,448]
    const_row = 1.0 + scale * k_c                    # [C]
    out_scale = -0.5 * (sp + D)
    return (
        np.ascontiguousarray(W1, dtype=np.float32),
        np.ascontiguousarray(W23, dtype=np.float32),
        np.ascontiguousarray(const_row, dtype=np.float32),
        np.ascontiguousarray(biases, dtype=np.float32),
        float(out_scale),
    )


def _pack_core_input(XqT_slice, W1, W23):
    """Per K-chunk: [XqT rows | W1[rows, 128c:] | W23 rows], flattened."""
    parts = []
    for c in range(KC):
        rows = slice(128 * c, 128 * (c + 1))
        block = np.concatenate(
            [XqT_slice[rows], W1[rows, 128 * c :], W23[rows]], axis=1
        )
        assert block.shape == (128, CHUNK_W[c])
        parts.append(block.ravel())
    out = np.concatenate(parts)
    assert out.shape == (INP_TOTAL,)
    return np.ascontiguousarray(out)


# --------------------------------------------------------------------------
# Bass kernel: per core, P = XqT.T @ Wbig then fused reductions + log.
# --------------------------------------------------------------------------
def _build_bass(out_scale):
    import concourse.tile as tile
    from concourse import bacc, mybir

    f32 = mybir.dt.float32
    f32r = mybir.dt.float32r
    nc = bacc.Bacc("TRN2", target_bir_lowering=False, debug=False)
    inp = nc.declare_dram_parameter("inp", [INP_TOTAL], f32r, isOutput=False)
    cb = nc.declare_dram_parameter("cb", [128, 2 * C], f32, isOutput=False)
    out = nc.declare_dram_parameter("out", [QLOC, C], f32, isOutput=True)

    with tile.TileContext(nc) as tc:
        with (
            tc.tile_pool(name="weights", bufs=1) as wpool,
            tc.tile_pool(name="acts", bufs=2) as apool,
            tc.tile_pool(name="scratch", bufs=2) as spool,
            tc.tile_pool(name="psum", bufs=2, space="PSUM") as ppool,
            tc.tile_pool(name="warm", bufs=1) as warmpool,
            tc.tile_pool(name="warmps", bufs=1, space="PSUM") as warmpspool,
        ):
            # --- PE warm-up: long garbage fp32 matmuls during the input DMA.
            wsrc = warmpool.tile([128, D], f32, tag="wsrc")
            nc.vector.memset(wsrc[:], 1.0)
            wps = warmpspool.tile([128, D], f32, tag="wps")
            for _ in range(N_WARM):
                nc.tensor.matmul(
                    wps[:], wsrc[:, 0:128], wsrc[:], start=True, stop=True
                )

            # --- inputs
            cb_sb = wpool.tile([128, 2 * C], f32, tag="cb")
            in_sb = []
            for c in range(KC):
                t = wpool.tile([128, CHUNK_W[c]], f32r, tag=f"in{c}")
                nc.sync.dma_start(
                    out=t[:],
                    in_=inp[CHUNK_OFF[c] : CHUNK_OFF[c] + 128 * CHUNK_W[c]].rearrange(
                        "(p w) -> p w", w=CHUNK_W[c]
                    ),
                )
                in_sb.append(t)
            nc.sync.dma_start(out=cb_sb[:], in_=cb[:, :])

            for qt in range(QT):
                ps = ppool.tile([128, NW], f32, tag="ps")
                for c in range(KC):
                    # A-part: upper-triangular W1, cols 128c:512
                    na = D - 128 * c
                    nc.tensor.matmul(
                        ps[:, 128 * c : D],
                        in_sb[c][:, qt * 128 : (qt + 1) * 128],
                        in_sb[c][:, QLOC : QLOC + na],
                        start=(c == 0),
                        stop=(c == KC - 1),
                    )
                for c in range(KC):
                    # B-part: W2 | W3, cols 512:960
                    na = D - 128 * c
                    nc.tensor.matmul(
                        ps[:, D:NW],
                        in_sb[c][:, qt * 128 : (qt + 1) * 128],
                        in_sb[c][:, QLOC + na : QLOC + na + NB],
                        start=(c == 0),
                        stop=(c == KC - 1),
                    )

                # t1 = rowsum(P[:, :512]^2)  (ScalarE: single PSUM input + accum)
                sq = spool.tile([128, D], f32, tag="sq")
                t1 = spool.tile([128, 1], f32, tag="t1")
                nc.scalar.activation(
                    out=sq[:],
                    in_=ps[:, 0:D],
                    func=mybir.ActivationFunctionType.Square,
                    accum_out=t1[:],
                )
                # s2 = group-of-6 rowsum(P[:, 576:960]^2)
                sq6 = spool.tile([128, C * RANK], f32, tag="sq6")
                nc.scalar.activation(
                    out=sq6[:],
                    in_=ps[:, D + C : NW],
                    func=mybir.ActivationFunctionType.Square,
                )
                s2 = spool.tile([128, C], f32, tag="s2")
                nc.vector.reduce_sum(
                    out=s2[:],
                    in_=sq6[:].rearrange("p (c s) -> p c s", s=RANK),
                    axis=mybir.AxisListType.X,
                )
                # u = T2 + const - s2 ; lg = ln(u + t1)
                u = spool.tile([128, C], f32, tag="u")
                nc.vector.tensor_sub(u[:], ps[:, D : D + C], s2[:])
                nc.vector.tensor_add(u[:], u[:], cb_sb[:, 0:C])
                lg = spool.tile([128, C], f32, tag="lg")
                nc.scalar.activation(
                    out=lg[:],
                    in_=u[:],
                    func=mybir.ActivationFunctionType.Ln,
                    bias=t1[:, 0:1],
                    scale=1.0,
                )
                # out = biases + out_scale * lg
                ot = apool.tile([128, C], f32, tag="ot")
                nc.vector.scalar_tensor_tensor(
                    out=ot[:],
                    in0=lg[:],
                    scalar=float(out_scale),
                    in1=cb_sb[:, C : 2 * C],
                    op0=mybir.AluOpType.mult,
                    op1=mybir.AluOpType.add,
                )
                nc.sync.dma_start(
                    out=out[qt * 128 : (qt + 1) * 128, :], in_=ot[:]
                )
    nc.compile()
    return nc


def kernel(X_support, y, X_query, m, kappa, nu, triu_S_diag, triu_S_lower):
    from concourse.bass_utils import run_bass_kernel_spmd

    W1, W23, const_row, biases, out_scale = _host_precompute(
        X_support, m, kappa, nu, triu_S_diag, triu_S_lower
    )
    Xq = np.ascontiguousarray(np.asarray(X_query, np.float32))
    XqT = np.ascontiguousarray(Xq.T)                 # [D, Q]
    cb_row = np.concatenate([const_row, biases])     # [2C]
    cb = np.ascontiguousarray(
        np.broadcast_to(cb_row[None, :], (128, 2 * C)), dtype=np.float32
    )

    in_maps = [
        {
            "inp": _pack_core_input(XqT[:, i * QLOC : (i + 1) * QLOC], W1, W23),
            "cb": cb,
        }
        for i in range(NCORES)
    ]
    nc = _build_bass(out_scale)
    trace = bool(int(os.environ.get("KBENCH_TRACE", "0")))
    res = run_bass_kernel_spmd(
        nc, in_maps, core_ids=list(range(NCORES)), trace=trace
    )
    if trace:
        kernel.last_exec_time_ns = res.exec_time_ns
        kernel.last_results = res
    out = np.concatenate([res.results[i]["out"] for i in range(NCORES)], axis=0)
    return out
